# revision 1
# baseline (speedup 1.0000x reference)
"""Trainium2 Bass kernel for nn_InteractionModule (GNN message passing).

Strategy (8 NeuronCores, SPMD, no collectives):
 - Nodes sharded 8 x 6250 by dst; edges assigned to the core owning dst,
   sorted by dst, grouped into 256-node chunks, padded to 128-edge subtiles
   (equal counts across cores so one NEFF serves all).
 - Each core computes the full table  yE = exp(ssp(x) @ W_diff.T + b_diff)
   into DRAM (replicated), then gathers yE rows per edge subtile with
   indirect DMA (one row per partition).
 - msg = ssp(y)*gate computed as ln(0.5*yE+0.5) * (ea @ G_w.T)  (ssp via
   Exp/Ln composite absorbs the -log2 exactly).
 - Segment-sum: per-subtile one-hot M (fp16, exact) against the chunk's
   node window; PE matmul msg.T @ M accumulates aggrT[f, node] in PSUM.
 - Update + 3 residual layers + out head computed in transposed [f, node]
   layout (weights pre-transposed on host), PE-transposed back to rows.
"""

import numpy as np

N, E, F, K, R = 50000, 600000, 128, 64, 3
NC_ = 8
NSH = N // NC_            # 6250 nodes per core
CHUNK = 256               # scatter window (one-hot width)
WIN = 512                 # phase-1/3 node window
LOG2 = float(np.log(2.0))
NCHUNK = (NSH + CHUNK - 1) // CHUNK   # 25
NWIN = (NSH + WIN - 1) // WIN         # 13

_cache = {}


def _prep(x, edge_index, edge_attr, W_diff, b_diff, G_w):
    """Host-side sharding: returns per-core edge arrays + structure lists."""
    src = np.asarray(edge_index[0], dtype=np.int64)
    dst = np.asarray(edge_index[1], dtype=np.int64)
    core = dst // NSH
    dstl = dst - core * NSH
    ea = np.asarray(edge_attr, dtype=np.float32)

    # per (core, chunk) edge lists
    chunk = dstl // CHUNK
    key = core * NCHUNK + chunk
    order = np.argsort(key, kind="stable")
    key_s = key[order]
    counts = np.bincount(key_s, minlength=NC_ * NCHUNK).reshape(NC_, NCHUNK)
    st = (counts + 127) // 128                      # subtiles per (core, chunk)
    st_max = st.max(axis=0)                         # equalized per chunk
    S = int(st_max.sum())
    S = ((S + 3) // 4) * 4                          # pad supertiles
    extra = S - int(st_max.sum())
    st_max[-1] += extra
    G = S // 4

    # chunk of each subtile + per-chunk first/last flags
    chunk_of_q = np.repeat(np.arange(NCHUNK), st_max)
    firsts = np.zeros(S, bool)
    lasts = np.zeros(S, bool)
    pos = 0
    for c in range(NCHUNK):
        firsts[pos] = True
        lasts[pos + st_max[c] - 1] = True
        pos += st_max[c]

    # build padded per-core flat edge arrays in (subtile, lane) order
    src_a = np.zeros((NC_, S * 128), np.int32)
    dstf_a = np.full((NC_, S * 128), -1.0, np.float32)
    ea_a = np.zeros((NC_, S * 128, K), np.float32)
    cum = np.zeros(NC_ * NCHUNK + 1, np.int64)
    np.cumsum(counts.ravel(), out=cum[1:])
    for c in range(NC_):
        pos = 0
        for ch in range(NCHUNK):
            k0 = cum[c * NCHUNK + ch]
            n_e = counts[c, ch]
            sl = order[k0 : k0 + n_e]
            src_a[c, pos : pos + n_e] = src[sl]
            dstf_a[c, pos : pos + n_e] = (dstl[sl] - ch * CHUNK).astype(np.float32)
            ea_a[c, pos : pos + n_e] = ea[sl]
            pos += int(st_max[ch]) * 128

    # device layouts
    # src/dstf: [128, S] column q = subtile, row p = lane
    src_d = src_a.reshape(NC_, S, 128).transpose(0, 2, 1).copy()
    dstf_d = dstf_a.reshape(NC_, S, 128).transpose(0, 2, 1).copy()
    # eaT packed: supertile pairs in upper/lower 64 partitions
    eaT = ea_a.reshape(NC_, G, 512, K).transpose(0, 1, 3, 2)  # [NC, G, K, 512]
    Gp = (G + 1) // 2
    ea_d = np.zeros((NC_, 128, Gp * 512), np.float16)
    ev = eaT[:, 0::2]
    ea_d[:, :K, : ev.shape[1] * 512] = ev.transpose(0, 2, 1, 3).reshape(NC_, K, -1)
    od = eaT[:, 1::2]
    ea_d[:, K:2 * K, : od.shape[1] * 512] = od.transpose(0, 2, 1, 3).reshape(NC_, K, -1)

    meta = dict(S=S, G=G, Gp=Gp,
                chunk_of_q=chunk_of_q.tolist(),
                firsts=firsts.tolist(), lasts=lasts.tolist())
    return src_d, dstf_d, ea_d, meta


def _build(nc, meta):
    import contextlib
    import concourse.bass as bass
    import concourse.mybir as mybir
    import concourse.tile as tile
    from concourse.masks import make_identity

    F32, F16, I32 = mybir.dt.float32, mybir.dt.float16, mybir.dt.int32
    AF, ALU = mybir.ActivationFunctionType, mybir.AluOpType
    S, G, Gp = meta["S"], meta["G"], meta["Gp"]
    cq, firsts, lasts = meta["chunk_of_q"], meta["firsts"], meta["lasts"]

    xT = nc.dram_tensor("xT", [F, N], F32, kind="ExternalInput").ap()
    wpack = nc.dram_tensor("wpack", [F, 9 * F], F32, kind="ExternalInput").ap()
    bpack = nc.dram_tensor("bpack", [F, 16], F32, kind="ExternalInput").ap()
    gw2 = nc.dram_tensor("gw2", [128, 128], F16, kind="ExternalInput").ap()
    iota_in = nc.dram_tensor("iota_in", [128, CHUNK], F16, kind="ExternalInput").ap()
    src_in = nc.dram_tensor("src_in", [128, S], I32, kind="ExternalInput").ap()
    dstf_in = nc.dram_tensor("dstf_in", [128, S], F32, kind="ExternalInput").ap()
    ea_in = nc.dram_tensor("ea_in", [128, Gp * 512], F16, kind="ExternalInput").ap()
    cid_in = nc.dram_tensor("cid_in", [1, 1], F32, kind="ExternalInput").ap()  # core id *NSH as float? unused on device path
    out0 = nc.dram_tensor("out0", [NSH, F], F32, kind="ExternalOutput").ap()
    out1 = nc.dram_tensor("out1", [NSH, F], F32, kind="ExternalOutput").ap()
    # own-shard xT slice passed separately so one NEFF serves all cores
    xTo = nc.dram_tensor("xTo", [F, NSH], F32, kind="ExternalInput").ap()

    NWIN_ALL = (N + WIN - 1) // WIN  # 98 windows over all nodes (N=50000 -> 97.65 -> 98)

    with tile.TileContext(nc) as tc, contextlib.ExitStack() as ctx:
        const = ctx.enter_context(tc.tile_pool(name="const", bufs=1))
        big = ctx.enter_context(tc.tile_pool(name="big", bufs=1))
        wk = ctx.enter_context(tc.tile_pool(name="wk", bufs=3))
        wk2 = ctx.enter_context(tc.tile_pool(name="wk2", bufs=3))
        ps_mm = ctx.enter_context(tc.tile_pool(name="psmm", bufs=2, space="PSUM"))
        ps_tr = ctx.enter_context(tc.tile_pool(name="pstr", bufs=2, space="PSUM"))
        ps_g = ctx.enter_context(tc.tile_pool(name="psg", bufs=2, space="PSUM"))
        ps_ag = ctx.enter_context(tc.tile_pool(name="psag", bufs=2, space="PSUM"))
        dram = ctx.enter_context(tc.tile_pool(name="dram", bufs=1, space="DRAM"))

        yE = dram.tile([N, F], F32)

        # ---- consts ----
        wp = const.tile([F, 9 * F], F32)
        nc.sync.dma_start(wp[:], wpack)
        bp = const.tile([F, 16], F32)
        nc.sync.dma_start(bp[:], bpack)
        gw = const.tile([128, 128], F16)
        nc.sync.dma_start(gw[:], gw2)
        iota = const.tile([128, CHUNK], F16)
        nc.sync.dma_start(iota[:], iota_in)
        srcs = const.tile([128, S], I32)
        nc.sync.dma_start(srcs[:], src_in)
        dstf = const.tile([128, S], F32)
        nc.sync.dma_start(dstf[:], dstf_in)
        ident = const.tile([128, 128], F32)
        make_identity(nc, ident[:])
        half = const.tile([128, 1], F32)
        nc.gpsimd.memset(half[:], 0.5)

        W_diffT = wp[:, 0:F]
        W_sameT = wp[:, F : 2 * F]
        W1T = [wp[:, (2 + i) * F : (3 + i) * F] for i in range(3)]
        W2T = [wp[:, (5 + i) * F : (6 + i) * F] for i in range(3)]
        W_lastT = wp[:, 8 * F : 9 * F]
        b_diff = bp[:, 0:1]
        b_same = bp[:, 1:2]
        b1 = [bp[:, 2 + i : 3 + i] for i in range(3)]
        b2 = [bp[:, 5 + i : 6 + i] for i in range(3)]
        b_last = bp[:, 8:9]
        uT = bp[:, 9:10]

        z_sT = big.tile([128, NSH], F32)
        xuT = big.tile([128, NSH], F32)

        # ---- phase 1: replicated yE table over all N nodes ----
        for w in range(NWIN_ALL):
            n0 = w * WIN
            nw = min(WIN, N - n0)
            xt = wk.tile([128, WIN], F32, tag="xt")
            nc.sync.dma_start(xt[:, :nw], xT[:, n0 : n0 + nw])
            ex = wk.tile([128, WIN], F32, tag="ex")
            nc.scalar.activation(ex[:, :nw], xt[:, :nw], AF.Exp)
            xa = wk.tile([128, WIN], F32, tag="xa")
            nc.scalar.activation(xa[:, :nw], ex[:, :nw], AF.Ln, bias=half[:, 0:1], scale=0.5)
            yps = ps_mm.tile([128, WIN], F32, tag="mm")
            nc.tensor.matmul(yps[:, :nw], W_diffT, xa[:, :nw], start=True, stop=True,
                             skip_group_check=True)
            yet = wk.tile([128, WIN], F32, tag="yet")
            nc.scalar.activation(yet[:, :nw], yps[:, :nw], AF.Exp, bias=b_diff)
            for j in range(0, nw, 128):
                pw = min(128, nw - j)
                tp = ps_tr.tile([128, 128], F32, tag="tp")
                nc.tensor.transpose(tp[:pw, :], yet[:, j : j + pw], ident[:])
                yr = wk2.tile([128, 128], F32, tag="yr")
                nc.vector.tensor_copy(yr[:pw, :], tp[:pw, :])
                nc.sync.dma_start(yE[n0 + j : n0 + j + pw, :], yr[:pw, :])

        # ---- phase 1b: own-shard z_same, xu ----
        for w in range(NWIN):
            n0 = w * WIN
            nw = min(WIN, NSH - n0)
            xt = wk.tile([128, WIN], F32, tag="xt")
            nc.sync.dma_start(xt[:, :nw], xTo[:, n0 : n0 + nw])
            nc.vector.tensor_scalar_mul(xuT[:, n0 : n0 + nw], xt[:, :nw], uT)
            ex = wk.tile([128, WIN], F32, tag="ex")
            nc.scalar.activation(ex[:, :nw], xt[:, :nw], AF.Exp)
            xa = wk.tile([128, WIN], F32, tag="xa")
            nc.scalar.activation(xa[:, :nw], ex[:, :nw], AF.Ln, bias=half[:, 0:1], scale=0.5)
            zps = ps_mm.tile([128, WIN], F32, tag="mm")
            nc.tensor.matmul(zps[:, :nw], W_sameT, xa[:, :nw], start=True, stop=True,
                             skip_group_check=True)
            nc.scalar.activation(z_sT[:, n0 : n0 + nw], zps[:, :nw], AF.Identity, bias=b_same)

        # ---- phase 2: edges ----
        aggr = None
        pend = []  # windows ready for phase 3
        done_chunks = 0

        def emit_phase3(w):
            n0 = w * WIN
            nw = min(WIN, NSH - n0)
            # out1 = msged_x rows
            for j in range(0, nw, 128):
                pw = min(128, nw - j)
                tp = ps_tr.tile([128, 128], F32, tag="tp")
                nc.tensor.transpose(tp[:pw, :], z_sT[:, n0 + j : n0 + j + pw], ident[:])
                orow = wk2.tile([128, 128], F32, tag="yr")
                nc.vector.tensor_copy(orow[:pw, :], tp[:pw, :])
                nc.sync.dma_start(out1[n0 + j : n0 + j + pw, :], orow[:pw, :])
            # residual stack (transposed layout)
            tw = wk2.tile([128, WIN], F32, tag="tw")
            cur = z_sT[:, n0 : n0 + nw]
            for i in range(R):
                e1 = wk.tile([128, WIN], F32, tag="ex")
                nc.scalar.activation(e1[:, :nw], cur, AF.Exp)
                s1 = wk.tile([128, WIN], F32, tag="xa")
                nc.scalar.activation(s1[:, :nw], e1[:, :nw], AF.Ln, bias=half[:, 0:1], scale=0.5)
                z1 = ps_mm.tile([128, WIN], F32, tag="mm")
                nc.tensor.matmul(z1[:, :nw], W1T[i], s1[:, :nw], start=True, stop=True,
                                 skip_group_check=True)
                e2 = wk.tile([128, WIN], F32, tag="ex")
                nc.scalar.activation(e2[:, :nw], z1[:, :nw], AF.Exp, bias=b1[i])
                s2 = wk.tile([128, WIN], F32, tag="xa")
                nc.scalar.activation(s2[:, :nw], e2[:, :nw], AF.Ln, bias=half[:, 0:1], scale=0.5)
                z2 = ps_mm.tile([128, WIN], F32, tag="mm")
                nc.tensor.matmul(z2[:, :nw], W2T[i], s2[:, :nw], start=True, stop=True,
                                 skip_group_check=True)
                nc.vector.scalar_tensor_tensor(tw[:, :nw], z2[:, :nw], b2[i], cur,
                                               ALU.add, ALU.add)
                cur = tw[:, :nw]
            ev = wk.tile([128, WIN], F32, tag="ex")
            nc.scalar.activation(ev[:, :nw], cur, AF.Exp)
            sv = wk.tile([128, WIN], F32, tag="xa")
            nc.scalar.activation(sv[:, :nw], ev[:, :nw], AF.Ln, bias=half[:, 0:1], scale=0.5)
            zv = ps_mm.tile([128, WIN], F32, tag="mm")
            nc.tensor.matmul(zv[:, :nw], W_lastT, sv[:, :nw], start=True, stop=True,
                             skip_group_check=True)
            o0t = wk2.tile([128, WIN], F32, tag="o0t")
            nc.vector.scalar_tensor_tensor(o0t[:, :nw], zv[:, :nw], b_last,
                                           xuT[:, n0 : n0 + nw], ALU.add, ALU.add)
            for j in range(0, nw, 128):
                pw = min(128, nw - j)
                tp = ps_tr.tile([128, 128], F32, tag="tp")
                nc.tensor.transpose(tp[:pw, :], o0t[:, j : j + pw], ident[:])
                orow = wk2.tile([128, 128], F32, tag="yr")
                nc.vector.tensor_copy(orow[:pw, :], tp[:pw, :])
                nc.sync.dma_start(out0[n0 + j : n0 + j + pw, :], orow[:pw, :])

        for g in range(G):
            yg = wk.tile([128, 4, F], F32, tag="yg")
            for t in range(4):
                q = 4 * g + t
                nc.gpsimd.indirect_dma_start(
                    out=yg[:, t, :], out_offset=None, in_=yE[:],
                    in_offset=bass.IndirectOffsetOnAxis(ap=srcs[:, q : q + 1], axis=0),
                )
            sp16 = wk.tile([128, 4, F], F16, tag="sp16")
            nc.scalar.activation(sp16[:].rearrange("p a b -> p (a b)"),
                                 yg[:].rearrange("p a b -> p (a b)"),
                                 AF.Ln, bias=half[:, 0:1], scale=0.5)
            gps = ps_g.tile([128, 4, F], F32, tag="gate")
            rb = 64 * (g % 2)
            cb = (g // 2) * 512
            eat = wk.tile([128, 512], F16, tag="eat")
            nc.sync.dma_start(eat[rb : rb + 64, :], ea_in[rb : rb + 64, cb : cb + 512])
            for t in range(4):
                nc.tensor.matmul(gps[:, t, :], eat[rb : rb + 64, 128 * t : 128 * t + 128],
                                 gw[rb : rb + 64, :], start=True, stop=True, skip_group_check=True)
            msg = wk.tile([128, 4, F], F16, tag="msg")
            nc.vector.tensor_tensor(msg[:].rearrange("p a b -> p (a b)"),
                                    sp16[:].rearrange("p a b -> p (a b)"),
                                    gps[:].rearrange("p a b -> p (a b)"), ALU.mult)
            for t in range(4):
                q = 4 * g + t
                c = cq[q]
                if firsts[q]:
                    aggr = ps_ag.tile([128, CHUNK], F32, tag="aggr")
                m16 = wk.tile([128, CHUNK], F16, tag="m16")
                nc.vector.tensor_scalar(m16[:], iota[:], dstf[:, q : q + 1], 0.0,
                                        ALU.subtract, ALU.is_equal)
                nc.tensor.matmul(aggr[:], msg[:, t, :], m16[:],
                                 start=bool(firsts[q]), stop=bool(lasts[q]),
                                 skip_group_check=True)
                if lasts[q]:
                    n0 = c * CHUNK
                    cw = min(CHUNK, NSH - n0)
                    ez = wk.tile([128, CHUNK], F32, tag="ez")
                    nc.scalar.activation(ez[:, :cw], z_sT[:, n0 : n0 + cw], AF.Exp)
                    spz = wk.tile([128, CHUNK], F32, tag="spz")
                    nc.scalar.activation(spz[:, :cw], ez[:, :cw], AF.Ln,
                                         bias=half[:, 0:1], scale=0.5)
                    nc.vector.tensor_tensor(z_sT[:, n0 : n0 + cw], spz[:, :cw],
                                            aggr[:, :cw], ALU.add)
                    done_chunks += 1
                    while done_chunks * CHUNK >= min((len(pend) + 1) * WIN, NSH) and len(pend) < NWIN:
                        w = len(pend)
                        pend.append(w)
                        emit_phase3(w)
        while len(pend) < NWIN:
            w = len(pend)
            pend.append(w)
            emit_phase3(w)
    return nc


def kernel(**inputs):
    import concourse.bacc as bacc
    from concourse import bass_utils

    x = np.asarray(inputs["x"], np.float32)
    W_diff = np.asarray(inputs["W_diff"], np.float32)
    b_diff = np.asarray(inputs["b_diff"], np.float32)
    G_w = np.asarray(inputs["G_w"], np.float32)
    src_d, dstf_d, ea_d, meta = _prep(x, inputs["edge_index"], inputs["edge_attr"],
                                      W_diff, b_diff, G_w)

    key = (meta["S"], meta["G"])
    if key not in _cache:
        nc = bacc.Bacc("TRN2", target_bir_lowering=False, debug=False,
                       enable_asserts=False, num_devices=NC_)
        _build(nc, meta)
        nc.compile()
        _cache[key] = nc
    nc = _cache[key]

    wpack = np.concatenate(
        [np.asarray(inputs[k], np.float32).T.copy() for k in ["W_diff", "W_same"]]
        + [np.asarray(inputs["res_W1"][i], np.float32).T.copy() for i in range(3)]
        + [np.asarray(inputs["res_W2"][i], np.float32).T.copy() for i in range(3)]
        + [np.asarray(inputs["W_last"], np.float32).T.copy()], axis=1)
    bpack = np.zeros((F, 16), np.float32)
    bpack[:, 0] = b_diff
    bpack[:, 1] = np.asarray(inputs["b_same"], np.float32)
    for i in range(3):
        bpack[:, 2 + i] = np.asarray(inputs["res_b1"][i], np.float32)
        bpack[:, 5 + i] = np.asarray(inputs["res_b2"][i], np.float32)
    bpack[:, 8] = np.asarray(inputs["b_last"], np.float32)
    bpack[:, 9] = np.asarray(inputs["u"], np.float32)[0]
    gw2 = np.zeros((128, 128), np.float16)
    gw2[:K] = G_w.T.astype(np.float16)
    gw2[64 : 64 + K] = G_w.T.astype(np.float16)
    iota = np.broadcast_to(np.arange(CHUNK, dtype=np.float16), (128, CHUNK)).copy()
    xT = x.T.copy()

    in_maps = []
    for c in range(NC_):
        in_maps.append(dict(
            xT=xT, wpack=wpack, bpack=bpack, gw2=gw2, iota_in=iota,
            src_in=src_d[c], dstf_in=dstf_d[c], ea_in=ea_d[c],
            cid_in=np.zeros((1, 1), np.float32),
            xTo=xT[:, c * NSH : (c + 1) * NSH].copy(),
        ))
    res = bass_utils.run_bass_kernel_spmd(nc, in_maps, core_ids=list(range(NC_)))
    o0 = np.concatenate([res.results[c]["out0"] for c in range(NC_)], axis=0)
    o1 = np.concatenate([res.results[c]["out1"] for c in range(NC_)], axis=0)
    return (o0, o1)

